# revision 5
# baseline (speedup 1.0000x reference)
"""Trainium2 Bass kernel for nn_BinarizeLayer (chain Viterbi binarization).

Algorithm (scaled formulation)
------------------------------
The reference is a 2-state Viterbi DP over an 8.4M-node chain.  With
d_i = (a0_i - a1_i)/(2*lam) (scaled score difference) the forward pass is

    d_i = e_i + clamp(d_{i-1}, -1/2, 1/2),    e_i = (2*p_i - 1)/(2*lam),

and backtracking is  label_{i-1} = (d_{i-1} + label_i > 1/2)  with labels
in {0,1}: this single comparison replaces the usual gt/ge bit pair
(label==1 needs d >= -1/2, label==0 needs d > 1/2; adding the label to d
before one fixed threshold realizes both).

Conjugating by prefix sums of ebar = -e:  s_k = sum_{j<=k} ebar_j,
w_k = d_k + s_k + 1/2 turns the clamp into scan-expressible forms:

    w_k = min(max(w_{k-1}, s_{k-1}), s_{k-1} + 1)      (tensor_tensor_scan)
    label_{k-1} = (w_k + label_k > s_k + 1)            (reversed scan)

Engine budget: tensor_tensor_scan only exists on the Vector engine
(codegen rejects it on Pool) and runs at ~2.1 ns/col regardless of
dtype, so scan columns are the scarce resource.  The walk and backtrack
scans are irreducible (sequential DP), but the prefix-sum scan is
decomposed into a G=8 reduce/recover tree of pairwise adds
(psum2/psum4/psum8 + a 1/8-length Vector scan + three strided recover
adds) whose tensor_tensor passes run on the otherwise-idle GpSimd
engine (~1.7 ns/col, no stride penalty).  ebar and SP = SB+1 run on the
Activation engine.  Resulting busy times per core: Vector ~38us,
GpSimd ~26us, Act ~16us, DMA ~16us.

Sharding: the chain is split into 8 core slices x 128 partition rows of
8192 payload elements; each row gets a 64-element halo on both sides
(the clamp walk and the backtrack both forget their initial state well
within 64 steps, so warm-up reproduces the exact sequential fp32 state).
The global chain ends are padded with p=0.5 (e=0 exactly), making the
boundary conditions exact: scan init 0.5 == d=0 before the first node,
and the reversed-scan init 0.5 implements the (d>0) final-label rule.
"""

import numpy as np

import concourse.bass as bass
import concourse.mybir as mybir
from concourse import tile
from concourse import bass_utils

LAM = 0.75
N = 8388608
NCORES = 8
P = 128          # partitions
W = 64           # halo / warm-up width
D = 8192         # payload elements per partition row
R = D + 2 * W    # row length incl. halos
ACH = 1040       # dma/act chunk width
CH = 2080        # compute chunk width (multiple of 8)
NCH = R // CH


def _build():
    f32 = mybir.dt.float32
    i8 = mybir.dt.int8
    Alu = mybir.AluOpType
    Copy = mybir.ActivationFunctionType.Copy

    nc = bass.Bass()
    x = nc.dram_tensor("x", [P, R], f32, kind="ExternalInput")
    y = nc.dram_tensor("y", [P, D], i8, kind="ExternalOutput")

    inv2l = 1.0 / (2.0 * LAM)

    with tile.TileContext(nc) as tc:
        with tc.tile_pool(name="big", bufs=1) as big:
            XT = big.tile([P, R], f32)         # input p, then ebar in place
            PS2 = big.tile([P, R // 2], f32)   # pair sums
            PS4 = big.tile([P, R // 4], f32)
            PS8 = big.tile([P, R // 8], f32)
            SB = big.tile([P, R + 1], f32)     # [j] = exclusive prefix sum
            SP = big.tile([P, R + 1], f32)     # SB + 1
            WT = big.tile([P, R], f32)         # walk values
            LB = big.tile([P, R], i8)          # labels ([W, R) valid)

            nc.gpsimd.memset(SB[:, 0:1], 0.0)
            nc.gpsimd.memset(SP[:, 0:1], 1.0)

            for c in range(R // ACH):
                a0, a1 = c * ACH, (c + 1) * ACH
                nc.sync.dma_start(XT[:, a0:a1], x[:, a0:a1])
                # ebar = (1 - 2p)/(2 lam), in place
                nc.scalar.activation(XT[:, a0:a1], XT[:, a0:a1],
                                     Copy, bias=inv2l, scale=-2.0 * inv2l)

            for c in range(NCH):
                c0, c1 = c * CH, (c + 1) * CH
                h2, h4, h8 = c0 // 2, c0 // 4, c0 // 8
                e2, e4, e8 = c1 // 2, c1 // 4, c1 // 8
                # reduce tree (GpSimd)
                nc.gpsimd.tensor_tensor(
                    PS2[:, h2:e2], XT[:, c0:c1:2], XT[:, c0 + 1:c1:2],
                    Alu.add)
                nc.gpsimd.tensor_tensor(
                    PS4[:, h4:e4], PS2[:, h2:e2:2], PS2[:, h2 + 1:e2:2],
                    Alu.add)
                nc.gpsimd.tensor_tensor(
                    PS8[:, h8:e8], PS4[:, h4:e4:2], PS4[:, h4 + 1:e4:2],
                    Alu.add)
                # 1/8-rate scan (Vector), chained across chunks
                nc.vector.tensor_tensor_scan(
                    SB[:, c0 + 8:c1 + 1:8], PS8[:, h8:e8], PS8[:, h8:e8],
                    0.0 if c0 == 0 else SB[:, c0:c0 + 1],
                    Alu.add, Alu.bypass)
                # recover exclusive sums at 4/2/1 offsets (GpSimd)
                nc.gpsimd.tensor_tensor(
                    SB[:, c0 + 4:c1:8], SB[:, c0:c1:8], PS4[:, h4:e4:2],
                    Alu.add)
                nc.gpsimd.tensor_tensor(
                    SB[:, c0 + 2:c1:4], SB[:, c0:c1:4], PS2[:, h2:e2:2],
                    Alu.add)
                nc.gpsimd.tensor_tensor(
                    SB[:, c0 + 1:c1:2], SB[:, c0:c1:2], XT[:, c0:c1:2],
                    Alu.add)
                # SP = SB + 1 (Act)
                nc.scalar.activation(SP[:, c0 + 1:c1 + 1],
                                     SB[:, c0 + 1:c1 + 1], Copy, bias=1.0)
                # walk scan (Vector), chained
                nc.vector.tensor_tensor_scan(
                    WT[:, c0:c1], SB[:, c0:c1], SP[:, c0:c1],
                    0.5 if c0 == 0 else WT[:, c0 - 1:c0],
                    Alu.max, Alu.min)

            # backtrack: reversed chained scans, right to left over [W, R)
            bounds = [W + i * ((R - W) // NCH) for i in range(NCH)] + [R]
            for c in range(NCH - 1, -1, -1):
                b0, b1 = bounds[c], bounds[c + 1]
                nc.vector.tensor_tensor_scan(
                    LB[:, b0:b1][:, ::-1],
                    WT[:, b0:b1][:, ::-1],
                    SP[:, b0 + 1:b1 + 1][:, ::-1],
                    0.5 if b1 == R else LB[:, b1:b1 + 1],
                    Alu.add, Alu.is_gt)
                nc.sync.dma_start(y[:, b0 - W:min(b1, W + D) - W],
                                  LB[:, b0:min(b1, W + D)])
    return nc


def _legalize_waits(nc, limit=1):
    """Split instructions carrying more than `limit` sem-waits.

    This walrus build rejects instructions whose sync_info has more wait
    commands than the ISA encoding allows (Tile can accumulate several).
    Excess waits move onto NoOps prepended on the same engine, which
    preserves per-engine ordering semantics.
    """
    import concourse.mybir as mybir
    for fn in nc.m.functions:
        for blk in fn.blocks:
            insts = blk.instructions
            i = 0
            while i < len(insts):
                inst = insts[i]
                si = getattr(inst, "sync_info", None)
                if si is not None and si.on_wait and len(si.on_wait) > limit:
                    waits = list(si.on_wait)
                    inst.sync_info = mybir.SyncInfo(
                        on_wait=waits[-limit:], on_update=list(si.on_update))
                    pending = waits[:-limit]
                    for j in range(0, len(pending), limit):
                        nop = mybir.InstNoOp(
                            name=nc.get_next_instruction_name(),
                            sync_info=mybir.SyncInfo(
                                on_wait=pending[j:j + limit], on_update=[]),
                            bass_nofuse=True,
                            engine=inst.engine,
                        )
                        insts.insert(i, nop)
                        i += 1
                i += 1
    return nc


_nc_cache = None


def _get_nc():
    global _nc_cache
    if _nc_cache is None:
        _nc_cache = _legalize_waits(_build())
    return _nc_cache


def _run(inputs: np.ndarray, **run_kwargs):
    p = np.ascontiguousarray(inputs, dtype=np.float32)
    assert p.shape == (N,)
    pad = np.full(W, 0.5, np.float32)
    pp = np.concatenate([pad, p, pad])
    nrows = N // D
    X = np.lib.stride_tricks.as_strided(pp, (nrows, R), (D * 4, 4))
    in_maps = [{"x": np.ascontiguousarray(X[k * P:(k + 1) * P])}
               for k in range(NCORES)]
    res = bass_utils.run_bass_kernel_spmd(_get_nc(), in_maps,
                                          core_ids=list(range(NCORES)),
                                          **run_kwargs)
    lab = np.concatenate([np.asarray(res.results[k]["y"]).reshape(-1)
                          for k in range(NCORES)])
    return lab.astype(np.int32), res


def kernel(inputs: np.ndarray) -> np.ndarray:
    return _run(inputs)[0]


# revision 13
# speedup vs baseline: 1.3315x; 1.3315x over previous
"""Trainium2 Bass kernel for nn_BinarizeLayer (chain Viterbi binarization).

Algorithm (scaled formulation)
------------------------------
The reference is a 2-state Viterbi DP over an 8.4M-node chain.  With
d_i = (a0_i - a1_i)/(2*lam) (scaled score difference) the forward pass is

    d_i = e_i + clamp(d_{i-1}, -1/2, 1/2),    e_i = (2*p_i - 1)/(2*lam),

and backtracking is  label_{i-1} = (d_{i-1} + label_i > 1/2)  with labels
in {0,1}: this single comparison replaces the usual gt/ge bit pair
(label==1 needs d >= -1/2, label==0 needs d > 1/2; adding the label to d
before one fixed threshold realizes both).

Conjugating by prefix sums of ebar = -e:  s_k = sum_{j<=k} ebar_j,
w_k = d_k + s_k + 1/2 turns the clamp into scan-expressible forms:

    w_k = min(max(w_{k-1}, s_{k-1}), s_{k-1} + 1)      (tensor_tensor_scan)
    label_{k-1} = (w_k + label_k > s_k + 1)            (reversed scan)

Engine notes (all measured on HW): tensor_tensor_scan exists only on the
Vector engine (codegen rejects Pool) and runs at ~2.1 ns/col regardless
of dtype or operand placement, so the three scans (sum, walk, backtrack
= ~25K columns/core) bound the kernel at ~53us of Vector busy time.
Offload attempts that fail: GpSimd tensor_tensor drops to ~4 ns/col
under concurrency and its strided writes race ahead of Vector readers
(wrong labels at chunk boundaries); PE prefix-sums need a column-major
layout whose transpose back is unaffordable.  ebar and SP = SB+1 run on
the Activation engine, overlapped chunk by chunk; chunks are chained
through the scan `initial` operand, so no intra-row warm-up is needed.

Sharding: the chain is split into 8 core slices x 128 partition rows of
8192 payload elements; each row gets a 64-element halo on both sides
(the clamp walk and the backtrack both forget their initial state well
within 64 steps, so warm-up reproduces the exact sequential fp32 state).
The global chain ends are padded with p=0.5 (e=0 exactly), making the
boundary conditions exact: scan init 0.5 == d=0 before the first node,
and the reversed-scan init 0.5 implements the (d>0) final-label rule.
"""

import numpy as np

import concourse.bass as bass
import concourse.mybir as mybir
from concourse import tile
from concourse import bass_utils

LAM = 0.75
N = 8388608
NCORES = 8
P = 128          # partitions
W = 64           # halo / warm-up width
D = 8192         # payload elements per partition row
R = D + 2 * W    # row length incl. halos

# forward chunks: small first chunk primes the DMA->ebar->scan pipeline
FWD = [1040, 2080, 2080, 2080, 1040]
BWD = [2064, 2064, 2064, 2064]      # backward chunks cover [W, R)
assert sum(FWD) == R and sum(BWD) == R - W


def _build():
    f32 = mybir.dt.float32
    i8 = mybir.dt.int8
    Alu = mybir.AluOpType
    Copy = mybir.ActivationFunctionType.Copy

    nc = bass.Bass()
    x = nc.dram_tensor("x", [P, R], f32, kind="ExternalInput")
    y = nc.dram_tensor("y", [P, D], i8, kind="ExternalOutput")

    inv2l = 1.0 / (2.0 * LAM)

    with tile.TileContext(nc) as tc:
        with tc.tile_pool(name="big", bufs=1) as big:
            XT = big.tile([P, R], f32)        # input p, then ebar in place
            SB = big.tile([P, R + 1], f32)    # [j] = exclusive prefix sum
            SP = big.tile([P, R + 1], f32)    # SB + 1
            WT = big.tile([P, R], f32)        # walk values
            LB = big.tile([P, R], i8)         # labels ([W, R) valid)

            nc.gpsimd.memset(SP[:, 0:1], 1.0)

            c0 = 0
            for ch in FWD:
                c1 = c0 + ch
                nc.sync.dma_start(XT[:, c0:c1], x[:, c0:c1])
                # ebar = (1 - 2p)/(2 lam), in place
                nc.scalar.activation(XT[:, c0:c1], XT[:, c0:c1],
                                     Copy, bias=inv2l, scale=-2.0 * inv2l)
                nc.vector.tensor_tensor_scan(
                    SB[:, c0 + 1:c1 + 1], XT[:, c0:c1], XT[:, c0:c1],
                    0.0 if c0 == 0 else SB[:, c0:c0 + 1],
                    Alu.add, Alu.bypass)
                nc.scalar.activation(SP[:, c0 + 1:c1 + 1],
                                     SB[:, c0 + 1:c1 + 1], Copy, bias=1.0)
                nc.vector.tensor_tensor_scan(
                    WT[:, c0:c1], SB[:, c0:c1], SP[:, c0:c1],
                    0.5 if c0 == 0 else WT[:, c0 - 1:c0],
                    Alu.max, Alu.min)
                c0 = c1

            # backtrack: reversed chained scans, right to left over [W, R)
            b1 = R
            for ch in BWD:
                b0 = b1 - ch
                nc.vector.tensor_tensor_scan(
                    LB[:, b0:b1][:, ::-1],
                    WT[:, b0:b1][:, ::-1],
                    SP[:, b0 + 1:b1 + 1][:, ::-1],
                    0.5 if b1 == R else LB[:, b1:b1 + 1],
                    Alu.add, Alu.is_gt)
                nc.sync.dma_start(y[:, b0 - W:min(b1, W + D) - W],
                                  LB[:, b0:min(b1, W + D)])
                b1 = b0
    return nc


def _legalize_waits(nc, limit=1):
    """Split instructions carrying more than `limit` sem-waits.

    This walrus build rejects instructions whose sync_info has more wait
    commands than the ISA encoding allows (Tile can accumulate several).
    Excess waits move onto NoOps prepended on the same engine, which
    preserves per-engine ordering semantics.
    """
    import concourse.mybir as mybir
    for fn in nc.m.functions:
        for blk in fn.blocks:
            insts = blk.instructions
            i = 0
            while i < len(insts):
                inst = insts[i]
                si = getattr(inst, "sync_info", None)
                if si is not None and si.on_wait and len(si.on_wait) > limit:
                    waits = list(si.on_wait)
                    inst.sync_info = mybir.SyncInfo(
                        on_wait=waits[-limit:], on_update=list(si.on_update))
                    pending = waits[:-limit]
                    for j in range(0, len(pending), limit):
                        nop = mybir.InstNoOp(
                            name=nc.get_next_instruction_name(),
                            sync_info=mybir.SyncInfo(
                                on_wait=pending[j:j + limit], on_update=[]),
                            bass_nofuse=True,
                            engine=inst.engine,
                        )
                        insts.insert(i, nop)
                        i += 1
                i += 1
    return nc


_nc_cache = None


def _get_nc():
    global _nc_cache
    if _nc_cache is None:
        _nc_cache = _legalize_waits(_build())
    return _nc_cache


def _run(inputs: np.ndarray, **run_kwargs):
    p = np.ascontiguousarray(inputs, dtype=np.float32)
    assert p.shape == (N,)
    pad = np.full(W, 0.5, np.float32)
    pp = np.concatenate([pad, p, pad])
    nrows = N // D
    X = np.lib.stride_tricks.as_strided(pp, (nrows, R), (D * 4, 4))
    in_maps = [{"x": np.ascontiguousarray(X[k * P:(k + 1) * P])}
               for k in range(NCORES)]
    res = bass_utils.run_bass_kernel_spmd(_get_nc(), in_maps,
                                          core_ids=list(range(NCORES)),
                                          **run_kwargs)
    lab = np.concatenate([np.asarray(res.results[k]["y"]).reshape(-1)
                          for k in range(NCORES)])
    return lab.astype(np.int32), res


def kernel(inputs: np.ndarray) -> np.ndarray:
    return _run(inputs)[0]


# revision 15
# speedup vs baseline: 1.5307x; 1.1495x over previous
"""Trainium2 Bass kernel for nn_BinarizeLayer (chain Viterbi binarization).

Algorithm (scaled formulation)
------------------------------
The reference is a 2-state Viterbi DP over an 8.4M-node chain.  With
d_i = (a0_i - a1_i)/(2*lam) (scaled score difference) the forward pass is

    d_i = e_i + clamp(d_{i-1}, -1/2, 1/2),    e_i = (2*p_i - 1)/(2*lam),

and backtracking is  label_{i-1} = (d_{i-1} + label_i > 1/2)  with labels
in {0,1}: this single comparison replaces the usual gt/ge bit pair
(label==1 needs d >= -1/2, label==0 needs d > 1/2; adding the label to d
before one fixed threshold realizes both).

Conjugating by prefix sums of ebar = -e:  s_k = sum_{j<=k} ebar_j,
w_k = d_k + s_k + 1/2 turns the clamp into scan-expressible forms:

    w_k = min(max(w_{k-1}, s_{k-1}), s_{k-1} + 1)      (tensor_tensor_scan)
    label_{k-1} = (w_k + label_k > s_k + 1)            (reversed scan)

Engine notes (all measured on HW): tensor_tensor_scan exists only on the
Vector engine (codegen rejects Pool) and runs at ~2.1 ns/col regardless
of dtype or operand placement, so the three scans (sum, walk, backtrack
= ~25K columns/core) bound the kernel at ~53us of Vector busy time.
Offload attempts that fail: GpSimd tensor_tensor drops to ~4 ns/col
under concurrency and its strided writes race ahead of Vector readers
(wrong labels at chunk boundaries); PE prefix-sums need a column-major
layout whose transpose back is unaffordable.  ebar and SP = SB+1 run on
the Activation engine, overlapped chunk by chunk; chunks are chained
through the scan `initial` operand, so no intra-row warm-up is needed.

Sharding: the chain is split into 8 core slices x 128 partition rows of
8192 payload elements; each row gets a 64-element halo on both sides
(the clamp walk and the backtrack both forget their initial state well
within 64 steps, so warm-up reproduces the exact sequential fp32 state).
The global chain ends are padded with p=0.5 (e=0 exactly), making the
boundary conditions exact: scan init 0.5 == d=0 before the first node,
and the reversed-scan init 0.5 implements the (d>0) final-label rule.
"""

import numpy as np

import concourse.bass as bass
import concourse.mybir as mybir
from concourse import tile
from concourse import bass_utils

LAM = 0.75
N = 8388608
NCORES = 8
P = 128          # partitions
W = 64           # halo / warm-up width
D = 8192         # payload elements per partition row
R = D + 2 * W    # row length incl. halos

# forward chunks: small first chunk primes the DMA->ebar->scan pipeline;
# 1040-wide chunks measure the best chained-scan rate (~2.25 ns/col)
FWD = [520, 520] + [1040] * 7
BWD = [1032] * 8                    # backward chunks cover [W, R)
assert sum(FWD) == R and sum(BWD) == R - W


def _build():
    f32 = mybir.dt.float32
    i8 = mybir.dt.int8
    Alu = mybir.AluOpType
    Copy = mybir.ActivationFunctionType.Copy

    nc = bass.Bass()
    x = nc.dram_tensor("x", [P, R], f32, kind="ExternalInput")
    y = nc.dram_tensor("y", [P, D], i8, kind="ExternalOutput")

    inv2l = 1.0 / (2.0 * LAM)

    with tile.TileContext(nc) as tc:
        with tc.tile_pool(name="big", bufs=1) as big:
            XT = big.tile([P, R], f32)        # input p, then ebar in place
            SB = big.tile([P, R + 1], f32)    # [j] = exclusive prefix sum
            SP = big.tile([P, R + 1], f32)    # SB + 1
            WT = big.tile([P, R], f32)        # walk values
            LB = big.tile([P, R], i8)         # labels ([W, R) valid)

            # on Vector: its consumer (walk chunk 0) is same-engine, which
            # avoids a Pool->Vector handoff and its semaphore
            nc.vector.memset(SP[:, 0:1], 1.0)

            c0 = 0
            for ch in FWD:
                c1 = c0 + ch
                nc.sync.dma_start(XT[:, c0:c1], x[:, c0:c1])
                # ebar = (1 - 2p)/(2 lam), in place
                nc.scalar.activation(XT[:, c0:c1], XT[:, c0:c1],
                                     Copy, bias=inv2l, scale=-2.0 * inv2l)
                nc.vector.tensor_tensor_scan(
                    SB[:, c0 + 1:c1 + 1], XT[:, c0:c1], XT[:, c0:c1],
                    0.0 if c0 == 0 else SB[:, c0:c0 + 1],
                    Alu.add, Alu.bypass)
                nc.scalar.activation(SP[:, c0 + 1:c1 + 1],
                                     SB[:, c0 + 1:c1 + 1], Copy, bias=1.0)
                nc.vector.tensor_tensor_scan(
                    WT[:, c0:c1], SB[:, c0:c1], SP[:, c0:c1],
                    0.5 if c0 == 0 else WT[:, c0 - 1:c0],
                    Alu.max, Alu.min)
                c0 = c1

            # backtrack: reversed chained scans, right to left over [W, R)
            b1 = R
            for ch in BWD:
                b0 = b1 - ch
                nc.vector.tensor_tensor_scan(
                    LB[:, b0:b1][:, ::-1],
                    WT[:, b0:b1][:, ::-1],
                    SP[:, b0 + 1:b1 + 1][:, ::-1],
                    0.5 if b1 == R else LB[:, b1:b1 + 1],
                    Alu.add, Alu.is_gt)
                nc.sync.dma_start(y[:, b0 - W:min(b1, W + D) - W],
                                  LB[:, b0:min(b1, W + D)])
                b1 = b0
    return nc


def _legalize_waits(nc, limit=1):
    """Split instructions carrying more than `limit` sem-waits.

    This walrus build rejects instructions whose sync_info has more wait
    commands than the ISA encoding allows (Tile can accumulate several).
    Excess waits move onto NoOps prepended on the same engine, which
    preserves per-engine ordering semantics.
    """
    import concourse.mybir as mybir
    for fn in nc.m.functions:
        for blk in fn.blocks:
            insts = blk.instructions
            i = 0
            while i < len(insts):
                inst = insts[i]
                si = getattr(inst, "sync_info", None)
                if si is not None and si.on_wait and len(si.on_wait) > limit:
                    waits = list(si.on_wait)
                    inst.sync_info = mybir.SyncInfo(
                        on_wait=waits[-limit:], on_update=list(si.on_update))
                    pending = waits[:-limit]
                    for j in range(0, len(pending), limit):
                        nop = mybir.InstNoOp(
                            name=nc.get_next_instruction_name(),
                            sync_info=mybir.SyncInfo(
                                on_wait=pending[j:j + limit], on_update=[]),
                            bass_nofuse=True,
                            engine=inst.engine,
                        )
                        insts.insert(i, nop)
                        i += 1
                i += 1
    return nc


_nc_cache = None


def _get_nc():
    global _nc_cache
    if _nc_cache is None:
        _nc_cache = _legalize_waits(_build())
    return _nc_cache


def _run(inputs: np.ndarray, **run_kwargs):
    p = np.ascontiguousarray(inputs, dtype=np.float32)
    assert p.shape == (N,)
    pad = np.full(W, 0.5, np.float32)
    pp = np.concatenate([pad, p, pad])
    nrows = N // D
    X = np.lib.stride_tricks.as_strided(pp, (nrows, R), (D * 4, 4))
    in_maps = [{"x": np.ascontiguousarray(X[k * P:(k + 1) * P])}
               for k in range(NCORES)]
    res = bass_utils.run_bass_kernel_spmd(_get_nc(), in_maps,
                                          core_ids=list(range(NCORES)),
                                          **run_kwargs)
    lab = np.concatenate([np.asarray(res.results[k]["y"]).reshape(-1)
                          for k in range(NCORES)])
    return lab.astype(np.int32), res


def kernel(inputs: np.ndarray) -> np.ndarray:
    return _run(inputs)[0]


# revision 16
# speedup vs baseline: 1.5330x; 1.0015x over previous
"""Trainium2 Bass kernel for nn_BinarizeLayer (chain Viterbi binarization).

Algorithm (scaled formulation)
------------------------------
The reference is a 2-state Viterbi DP over an 8.4M-node chain.  With
d_i = (a0_i - a1_i)/(2*lam) (scaled score difference) the forward pass is

    d_i = e_i + clamp(d_{i-1}, -1/2, 1/2),    e_i = (2*p_i - 1)/(2*lam),

and backtracking is  label_{i-1} = (d_{i-1} + label_i > 1/2)  with labels
in {0,1}: this single comparison replaces the usual gt/ge bit pair
(label==1 needs d >= -1/2, label==0 needs d > 1/2; adding the label to d
before one fixed threshold realizes both).

Conjugating by prefix sums of ebar = -e:  s_k = sum_{j<=k} ebar_j,
w_k = d_k + s_k + 1/2 turns the clamp into scan-expressible forms:

    w_k = min(max(w_{k-1}, s_{k-1}), s_{k-1} + 1)      (tensor_tensor_scan)
    label_{k-1} = (w_k + label_k > s_k + 1)            (reversed scan)

Engine notes (all measured on HW): tensor_tensor_scan exists only on the
Vector engine (codegen rejects Pool) and runs at ~2.1 ns/col regardless
of dtype or operand placement, so the three scans (sum, walk, backtrack
= ~25K columns/core) bound the kernel at ~53us of Vector busy time.
Offload attempts that fail: GpSimd tensor_tensor drops to ~4 ns/col
under concurrency and its strided writes race ahead of Vector readers
(wrong labels at chunk boundaries); PE prefix-sums need a column-major
layout whose transpose back is unaffordable.  ebar and SP = SB+1 run on
the Activation engine, overlapped chunk by chunk; chunks are chained
through the scan `initial` operand, so no intra-row warm-up is needed.

Sharding: the chain is split into 8 core slices x 128 partition rows of
8192 payload elements; each row gets a 64-element halo on both sides
(the clamp walk and the backtrack both forget their initial state well
within 64 steps, so warm-up reproduces the exact sequential fp32 state).
The global chain ends are padded with p=0.5 (e=0 exactly), making the
boundary conditions exact: scan init 0.5 == d=0 before the first node,
and the reversed-scan init 0.5 implements the (d>0) final-label rule.
"""

import numpy as np

import concourse.bass as bass
import concourse.mybir as mybir
from concourse import tile
from concourse import bass_utils

LAM = 0.75
N = 8388608
NCORES = 8
P = 128          # partitions
W = 64           # halo / warm-up width
D = 8192         # payload elements per partition row
R = D + 2 * W    # row length incl. halos

# forward chunks: small first chunk primes the DMA->ebar->scan pipeline;
# 1040-wide chunks measure the best chained-scan rate (~2.25 ns/col)
FWD = [520, 520] + [1040] * 7
BWD = [1032] * 8                    # backward chunks cover [W, R)
assert sum(FWD) == R and sum(BWD) == R - W


def _build():
    f32 = mybir.dt.float32
    i8 = mybir.dt.int8
    Alu = mybir.AluOpType
    Copy = mybir.ActivationFunctionType.Copy

    nc = bass.Bass()
    x = nc.dram_tensor("x", [P, R], f32, kind="ExternalInput")
    y = nc.dram_tensor("y", [P, D], i8, kind="ExternalOutput")

    inv2l = 1.0 / (2.0 * LAM)

    with tile.TileContext(nc) as tc:
        with tc.tile_pool(name="big", bufs=1) as big:
            XT = big.tile([P, R], f32)        # input p, then ebar in place
            SB = big.tile([P, R + 1], f32)    # [j] = exclusive prefix sum
            SP = big.tile([P, R + 1], f32)    # SB + 1
            WT = big.tile([P, R], f32)        # walk values
            LB = big.tile([P, R], i8)         # labels ([W, R) valid)

            # on Vector: its consumer (walk chunk 0) is same-engine, which
            # avoids a Pool->Vector handoff and its semaphore
            nc.vector.memset(SP[:, 0:1], 1.0)

            # phase-major emission: per-engine instruction order follows
            # program order, so interleaving sum/SP/walk per chunk makes
            # the walk wait out Act's SP latency every chunk (~1-2us gaps).
            # Grouping all sums first lets Act compute every SP while the
            # sum phase still runs; the walk phase then has zero stalls.
            edges = [0]
            for ch in FWD:
                edges.append(edges[-1] + ch)
            for c0, c1 in zip(edges, edges[1:]):
                nc.sync.dma_start(XT[:, c0:c1], x[:, c0:c1])
                # ebar = (1 - 2p)/(2 lam), in place
                nc.scalar.activation(XT[:, c0:c1], XT[:, c0:c1],
                                     Copy, bias=inv2l, scale=-2.0 * inv2l)
            for c0, c1 in zip(edges, edges[1:]):
                nc.vector.tensor_tensor_scan(
                    SB[:, c0 + 1:c1 + 1], XT[:, c0:c1], XT[:, c0:c1],
                    0.0 if c0 == 0 else SB[:, c0:c0 + 1],
                    Alu.add, Alu.bypass)
            for c0, c1 in zip(edges, edges[1:]):
                nc.scalar.activation(SP[:, c0 + 1:c1 + 1],
                                     SB[:, c0 + 1:c1 + 1], Copy, bias=1.0)
            for c0, c1 in zip(edges, edges[1:]):
                nc.vector.tensor_tensor_scan(
                    WT[:, c0:c1], SB[:, c0:c1], SP[:, c0:c1],
                    0.5 if c0 == 0 else WT[:, c0 - 1:c0],
                    Alu.max, Alu.min)

            # backtrack: reversed chained scans, right to left over [W, R)
            b1 = R
            for ch in BWD:
                b0 = b1 - ch
                nc.vector.tensor_tensor_scan(
                    LB[:, b0:b1][:, ::-1],
                    WT[:, b0:b1][:, ::-1],
                    SP[:, b0 + 1:b1 + 1][:, ::-1],
                    0.5 if b1 == R else LB[:, b1:b1 + 1],
                    Alu.add, Alu.is_gt)
                nc.sync.dma_start(y[:, b0 - W:min(b1, W + D) - W],
                                  LB[:, b0:min(b1, W + D)])
                b1 = b0
    return nc


def _legalize_waits(nc, limit=1):
    """Split instructions carrying more than `limit` sem-waits.

    This walrus build rejects instructions whose sync_info has more wait
    commands than the ISA encoding allows (Tile can accumulate several).
    Excess waits move onto NoOps prepended on the same engine, which
    preserves per-engine ordering semantics.
    """
    import concourse.mybir as mybir
    for fn in nc.m.functions:
        for blk in fn.blocks:
            insts = blk.instructions
            i = 0
            while i < len(insts):
                inst = insts[i]
                si = getattr(inst, "sync_info", None)
                if si is not None and si.on_wait and len(si.on_wait) > limit:
                    waits = list(si.on_wait)
                    inst.sync_info = mybir.SyncInfo(
                        on_wait=waits[-limit:], on_update=list(si.on_update))
                    pending = waits[:-limit]
                    for j in range(0, len(pending), limit):
                        nop = mybir.InstNoOp(
                            name=nc.get_next_instruction_name(),
                            sync_info=mybir.SyncInfo(
                                on_wait=pending[j:j + limit], on_update=[]),
                            bass_nofuse=True,
                            engine=inst.engine,
                        )
                        insts.insert(i, nop)
                        i += 1
                i += 1
    return nc


_nc_cache = None


def _get_nc():
    global _nc_cache
    if _nc_cache is None:
        _nc_cache = _legalize_waits(_build())
    return _nc_cache


def _run(inputs: np.ndarray, **run_kwargs):
    p = np.ascontiguousarray(inputs, dtype=np.float32)
    assert p.shape == (N,)
    pad = np.full(W, 0.5, np.float32)
    pp = np.concatenate([pad, p, pad])
    nrows = N // D
    X = np.lib.stride_tricks.as_strided(pp, (nrows, R), (D * 4, 4))
    in_maps = [{"x": np.ascontiguousarray(X[k * P:(k + 1) * P])}
               for k in range(NCORES)]
    res = bass_utils.run_bass_kernel_spmd(_get_nc(), in_maps,
                                          core_ids=list(range(NCORES)),
                                          **run_kwargs)
    lab = np.concatenate([np.asarray(res.results[k]["y"]).reshape(-1)
                          for k in range(NCORES)])
    return lab.astype(np.int32), res


def kernel(inputs: np.ndarray) -> np.ndarray:
    return _run(inputs)[0]
